# revision 36
# baseline (speedup 1.0000x reference)
"""GAT message-passing kernel for 8 Trainium2 NeuronCores.

Key algebraic property of the reference (faithful torch repeat_interleave
replication): with h = x @ proj_w.T + proj_b  [B, N, H],
    first[b, I, J, c]  = h[b, I, (J*H+c) // N] = h[b, I, J // (N//H)]
    second[b, I, J, c] = h[b, I, c]
so the pre-mask score collapses to
    scores[b, I, J] = leaky_relu(S1 * h[b, I, J//32] + d[b, I])
with S1 = sum(a_w[0, :H]) and d = h @ a_w[0, H:].  Each row of scores has
only H=32 distinct values (one per 32-column block of J).  Softmax+matmul
then reduce to a masked weighted aggregation that never materializes any
[N, N] tensor in HBM:
    W[b, I, J] = adj[I, J] * exp(leaky(v))[b, I, J//32]
    out[b, I, :] = (W @ h[b]) / rowsum(W)

Sharding: rows I are split 128-per-core across 8 cores (both batches on
every core).  dist_mat rows are sharded; x and the tiny weights are
replicated.

Two structural tricks shrink device work to a handful of PE ops:

1. J-side tiling J = 8*q + tk (q = partition, tk = tile 0..7): x loads
   become fully contiguous and k(J) = J//32 = q//4 for every tile, so the
   score-broadcast tile eb[q, I] = e[q//4, I] is shared by all 8 J-tiles
   (built once per batch: an indicator matmul for batch 0, a replicate
   DMA for batch 1 whose latency hides under batch-0 work).

2. Aggregation associativity: out = (W @ x_aug) @ wt_aug, with
   x_aug = [x | 1].  The kernel accumulates GT[cc, I] = sum_J
   x_aug[J, cc] * W.T[J, I] with PSUM-accumulated matmuls whose lhsT is
   the *natural-layout* x_aug tile — h is never materialized and x is
   never transposed.  GT[64, :] = Z (softmax denominator).  One final
   [65, 33] matmul per batch applies wt_aug (bias row included; its ones
   column carries Z into column 32 of the output); the divide by Z
   happens on host on the [B, N, 33] result.

adjT tiles come from PE transposes of stride-8 column slices of the
core's dist rows (diagonal pre-zeroed on host => forced diag-1), compared
against the threshold on DVE.  W.T tiles are adjT * eb elementwise.  The
serial v-path (scores, one fused [32, 2*128] chain for both batches) is
interleaved with the adj transposes so PE never idles, and GT
accumulation alternates two PSUM banks to pipeline the fp32 passes.
"""

import sys

sys.path.insert(0, "/opt/trn_rl_repo")

import numpy as np

B, N, C, H = 2, 1024, 64, 32
P = 128                 # rows per core / partition tile
NCORES = 8
NJ = N // P             # 8 J-tiles of 128
THR = 200000.0
ALPHA = 0.01
H1 = H + 1              # 33: h channels + ones column
C1 = C + 1              # 65: x channels + ones column

_CACHE = {}
LAST_RESULT = None


def _build():
    import concourse.bacc as bacc
    import concourse.bass as bass
    import concourse.tile as tile
    from concourse import masks, mybir

    F32 = mybir.dt.float32
    Alu = mybir.AluOpType
    Act = mybir.ActivationFunctionType

    nc = bacc.Bacc("TRN2", target_bir_lowering=False)

    xa_d = nc.dram_tensor("xa", (B, P, NJ * C1), F32, kind="ExternalInput")
    xoT_d = nc.dram_tensor("xoT", (C, B * P), F32, kind="ExternalInput")
    distT_d = nc.dram_tensor("distT", (N, P), F32, kind="ExternalInput")
    # packed consts: cols 0:32 wm (rows 0:64), 32:33 vcol (rows 0:32),
    # 64:192 ind (rows 0:32), 192:225 wfin (rows 0:65)
    cblk_d = nc.dram_tensor("cblk", (P, 256), F32, kind="ExternalInput")
    # un-normalized output + Z column; the divide happens on host
    out_d = nc.dram_tensor("out", (B, P, H1), F32, kind="ExternalOutput")

    with tile.TileContext(nc) as tc:
        with (
            tc.tile_pool(name="const", bufs=1) as const,
            tc.tile_pool(name="persist", bufs=1) as persist,
            tc.tile_pool(name="work", bufs=3) as work,
            tc.tile_pool(name="psT", bufs=2, space="PSUM") as psT,
            tc.tile_pool(name="psE", bufs=2, space="PSUM") as psE,
            tc.tile_pool(name="psA", bufs=2, space="PSUM") as psA,
        ):
            # ---- input DMAs: all large loads are fully contiguous ----
            cblk = const.tile([P, 256], F32)
            nc.scalar.dma_start(out=cblk[:], in_=cblk_d[:])
            wm = cblk[0:C, 0:H]
            vcol = cblk[0:H, H : H + 1]
            ind = cblk[0:H, 64:192]
            wfin = cblk[0:C1, 192 : 192 + H1]
            xoT_sb = const.tile([C, B * P], F32)
            nc.scalar.dma_start(out=xoT_sb[:], in_=xoT_d[:])
            # distT rows J = 8q + tk: partition q reads 4KB contiguous
            dT_sb = persist.tile([P, NJ, P], F32)
            dTview = distT_d[:].rearrange("(q t) i -> q t i", t=NJ)
            nc.sync.dma_start(out=dT_sb[:, 0:4, :], in_=dTview[:, 0:4, :])
            xa_sb = persist.tile([P, B, NJ * C1], F32)
            nc.sync.dma_start(out=xa_sb[:, 0, :], in_=xa_d[0])
            nc.sync.dma_start(out=dT_sb[:, 4:8, :], in_=dTview[:, 4:8, :])
            nc.scalar.dma_start(out=xa_sb[:, 1, :], in_=xa_d[1])

            ebst = {}

            eb1_sb = persist.tile([P, P], F32)

            def v_steps():
                # both batches in one [32, 256] chain
                ps_v = psT.tile([H, B * P], F32, tag="ps")
                nc.tensor.matmul(ps_v[:], wm, xoT_sb[:])
                t2 = work.tile([H, B * P], F32, tag="t2")
                nc.scalar.activation(
                    t2[:], ps_v[:], Act.Lrelu, bias=vcol, alpha=ALPHA
                )
                eT = work.tile([H, B * P], F32, tag="eT")
                nc.scalar.activation(eT[:], t2[:], Act.Exp)
                ebst["e"] = eT
                # b=1 broadcast via replicate DMA (latency hidden by b0 work)
                src = eT[:, P : 2 * P]
                rep = bass.AP(
                    tensor=src.tensor, offset=src.offset,
                    ap=[list(src.ap[0]), [0, 4], list(src.ap[1])],
                )
                nc.sync.dma_start(out=eb1_sb[:], in_=rep)
                ebst[1] = eb1_sb

            def eb_mm():
                # b=0 broadcast eb[q, I] = e[q//4, I] via indicator matmul
                ps_eb = psE.tile([P, P], F32, tag="eb", name="eb0")
                nc.tensor.matmul(ps_eb[:], ind, ebst["e"][:, 0:P])
                ebst[0] = ps_eb

            aggst = {}

            def agg_step(b, tk):
                if tk == 0:
                    aggst[b] = (
                        psA.tile([C1, P], F32, tag="pse", name=f"pse{b}"),
                        psA.tile([C1, P], F32, tag="pso", name=f"pso{b}"),
                    )
                wtile = work.tile([P, P], F32, tag="wtile")
                nc.vector.scalar_tensor_tensor(
                    out=wtile[:], in0=dT_sb[:, tk, :], scalar=THR,
                    in1=ebst[b][:], op0=Alu.is_lt, op1=Alu.mult,
                )
                ps_g = aggst[b][tk % 2]
                nc.tensor.matmul(
                    ps_g[:],
                    xa_sb[:, b, C1 * tk : C1 * tk + C1],
                    wtile[:],
                    start=(tk < 2),
                    stop=(tk >= NJ - 2),
                    skip_group_check=True,
                )

            def finalize(b):
                ps_even, ps_odd = aggst[b]
                gtmp = work.tile([C1, P], F32, tag="gtmp")
                nc.vector.tensor_copy(gtmp[:], ps_odd[:])
                gt = work.tile([C1, P], F32, tag="gt")
                nc.vector.tensor_add(out=gt[:], in0=ps_even[:], in1=gtmp[:])
                ps_f = psT.tile([P, H1], F32, tag="ps", name=f"psf{b}")
                nc.tensor.matmul(ps_f[:], gt[:], wfin)
                ot = work.tile([P, H1], F32, tag="ot", name=f"ot{b}")
                nc.vector.tensor_copy(ot[:], ps_f[:])
                eng = nc.sync if b == 0 else nc.scalar
                eng.dma_start(out=out_d[b], in_=ot[:])

            # ---- interleaved emission ----
            v_steps()
            eb_mm()
            agg_step(0, 0)
            agg_step(0, 1)
            agg_step(0, 2)
            agg_step(0, 3)
            agg_step(0, 4)
            agg_step(0, 5)
            agg_step(0, 6)
            agg_step(0, 7)
            agg_step(1, 0)
            agg_step(1, 1)
            finalize(0)
            for tk in range(2, NJ):
                agg_step(1, tk)
            finalize(1)

    nc.finalize()
    return nc


def kernel(x, dist_mat, proj_w, proj_b, a_w, trace=False):
    global LAST_RESULT
    from concourse.bass_utils import run_bass_kernel_spmd

    x = np.ascontiguousarray(np.asarray(x, dtype=np.float32))
    dist_mat = np.asarray(dist_mat, dtype=np.float32)
    proj_w = np.asarray(proj_w, dtype=np.float32)
    proj_b = np.asarray(proj_b, dtype=np.float32).reshape(H)
    a_w = np.asarray(a_w, dtype=np.float32).reshape(2 * H)

    if "nc" not in _CACHE:
        _CACHE["nc"] = _build()
    nc = _CACHE["nc"]

    # ---- host-side constant folding (all tiny) ----
    a1, a2 = a_w[:H], a_w[H:]
    s1 = np.float32(a1.sum(dtype=np.float32))
    m32 = s1 * np.eye(H, dtype=np.float32) + a2[:, None]  # v = m32.T @ hT
    wta = proj_w.T.astype(np.float32)                     # [C, H]
    wm = np.ascontiguousarray(wta @ m32)                  # fold h->v projection
    vcol = np.ascontiguousarray((m32.T @ proj_b).reshape(H, 1))
    # final projection [x | 1] -> [h | 1]: bias row, ones column slot
    wfin = np.zeros((C1, H1), np.float32)
    wfin[:C, :H] = wta
    wfin[C, :H] = proj_b
    wfin[C, H] = 1.0
    # eb indicator: ind[k, q] = 1 iff q//4 == k
    ind = np.zeros((H, P), np.float32)
    for k in range(H):
        ind[k, 4 * k : 4 * k + 4] = 1.0
    # pack all the small constants into one block -> a single DMA
    cblk0 = np.zeros((P, 256), np.float32)
    cblk0[0:C, 0:H] = wm
    cblk0[0:H, H : H + 1] = vcol
    cblk0[0:H, 64:192] = ind
    cblk0[0:C1, 192 : 192 + H1] = wfin

    dist_fixed = dist_mat.copy()
    np.fill_diagonal(dist_fixed, 0.0)  # adj diagonal forced to 1
    distT = np.ascontiguousarray(dist_fixed.T)

    # token J = 8*q + tk; row q of xa[b] holds tokens 8q..8q+7, each
    # augmented with a trailing 1.0 (for the softmax denominator)
    xa = np.ones((B, N, C1), np.float32)
    xa[:, :, :C] = x
    xa = xa.reshape(B, P, NJ * C1)

    in_maps = []
    for c in range(NCORES):
        sl = slice(c * P, (c + 1) * P)
        in_maps.append(
            {
                "xa": xa,
                "xoT": np.concatenate([x[0, sl, :].T, x[1, sl, :].T], axis=1),
                "distT": np.ascontiguousarray(distT[:, sl]),
                "cblk": cblk0,
            }
        )

    res = run_bass_kernel_spmd(nc, in_maps, core_ids=list(range(NCORES)), trace=trace)
    LAST_RESULT = res
    full = np.concatenate([res.results[c]["out"] for c in range(NCORES)], axis=1)
    return np.ascontiguousarray(full[:, :, :H] / full[:, :, H : H + 1])


# revision 37
# speedup vs baseline: 1.0337x; 1.0337x over previous
"""GAT message-passing kernel for 8 Trainium2 NeuronCores.

Key algebraic property of the reference (faithful torch repeat_interleave
replication): with h = x @ proj_w.T + proj_b  [B, N, H],
    first[b, I, J, c]  = h[b, I, (J*H+c) // N] = h[b, I, J // (N//H)]
    second[b, I, J, c] = h[b, I, c]
so the pre-mask score collapses to
    scores[b, I, J] = leaky_relu(S1 * h[b, I, J//32] + d[b, I])
with S1 = sum(a_w[0, :H]) and d = h @ a_w[0, H:].  Each row of scores has
only H=32 distinct values (one per 32-column block of J).  Softmax+matmul
then reduce to a masked weighted aggregation that never materializes any
[N, N] tensor in HBM:
    W[b, I, J] = adj[I, J] * exp(leaky(v))[b, I, J//32]
    out[b, I, :] = (W @ h[b]) / rowsum(W)

Sharding: rows I are split 128-per-core across 8 cores (both batches on
every core).  dist_mat rows are sharded; x and the tiny weights are
replicated.

Two structural tricks shrink device work to a handful of PE ops:

1. J-side tiling J = 8*q + tk (q = partition, tk = tile 0..7): x loads
   become fully contiguous and k(J) = J//32 = q//4 for every tile, so the
   score-broadcast tile eb[q, I] = e[q//4, I] is shared by all 8 J-tiles
   (built once per batch: an indicator matmul for batch 0, a replicate
   DMA for batch 1 whose latency hides under batch-0 work).

2. Aggregation associativity: out = (W @ x_aug) @ wt_aug, with
   x_aug = [x | 1].  The kernel accumulates GT[cc, I] = sum_J
   x_aug[J, cc] * W.T[J, I] with PSUM-accumulated matmuls whose lhsT is
   the *natural-layout* x_aug tile — h is never materialized and x is
   never transposed.  GT[64, :] = Z (softmax denominator).  One final
   [65, 33] matmul per batch applies wt_aug (bias row included; its ones
   column carries Z into column 32 of the output); the divide by Z
   happens on host on the [B, N, 33] result.

adjT tiles come from PE transposes of stride-8 column slices of the
core's dist rows (diagonal pre-zeroed on host => forced diag-1), compared
against the threshold on DVE.  W.T tiles are adjT * eb elementwise.  The
serial v-path (scores, one fused [32, 2*128] chain for both batches) is
interleaved with the adj transposes so PE never idles, and GT
accumulation alternates two PSUM banks to pipeline the fp32 passes.
"""

import sys

sys.path.insert(0, "/opt/trn_rl_repo")

import numpy as np

B, N, C, H = 2, 1024, 64, 32
P = 128                 # rows per core / partition tile
NCORES = 8
NJ = N // P             # 8 J-tiles of 128
THR = 200000.0
ALPHA = 0.01
H1 = H + 1              # 33: h channels + ones column
C1 = C + 1              # 65: x channels + ones column

_CACHE = {}
LAST_RESULT = None


def _build():
    import concourse.bacc as bacc
    import concourse.bass as bass
    import concourse.tile as tile
    from concourse import masks, mybir

    F32 = mybir.dt.float32
    Alu = mybir.AluOpType
    Act = mybir.ActivationFunctionType

    nc = bacc.Bacc("TRN2", target_bir_lowering=False)

    xa_d = nc.dram_tensor("xa", (B, P, NJ * C1), F32, kind="ExternalInput")
    xoT_d = nc.dram_tensor("xoT", (C, B * P), F32, kind="ExternalInput")
    distT_d = nc.dram_tensor("distT", (N, P), F32, kind="ExternalInput")
    # packed consts: cols 0:32 wm (rows 0:64), 32:33 vcol (rows 0:32),
    # 64:192 ind (rows 0:32), 192:225 wfin (rows 0:65)
    cblk_d = nc.dram_tensor("cblk", (P, 256), F32, kind="ExternalInput")
    # un-normalized output + Z column; the divide happens on host
    out_d = nc.dram_tensor("out", (B, P, H1), F32, kind="ExternalOutput")

    with tile.TileContext(nc) as tc:
        with (
            tc.tile_pool(name="const", bufs=1) as const,
            tc.tile_pool(name="persist", bufs=1) as persist,
            tc.tile_pool(name="work", bufs=3) as work,
            tc.tile_pool(name="psT", bufs=2, space="PSUM") as psT,
            tc.tile_pool(name="psE", bufs=2, space="PSUM") as psE,
            tc.tile_pool(name="psA", bufs=2, space="PSUM") as psA,
        ):
            # ---- input DMAs: all large loads are fully contiguous ----
            cblk = const.tile([P, 256], F32)
            nc.scalar.dma_start(out=cblk[:], in_=cblk_d[:])
            wm = cblk[0:C, 0:H]
            vcol = cblk[0:H, H : H + 1]
            ind = cblk[0:H, 64:192]
            wfin = cblk[0:C1, 192 : 192 + H1]
            xoT_sb = const.tile([C, B * P], F32)
            nc.scalar.dma_start(out=xoT_sb[:], in_=xoT_d[:])
            # distT rows J = 8q + tk: partition q reads 4KB contiguous
            dT_sb = persist.tile([P, NJ, P], F32)
            dTview = distT_d[:].rearrange("(q t) i -> q t i", t=NJ)
            nc.sync.dma_start(out=dT_sb[:, 0:4, :], in_=dTview[:, 0:4, :])
            xa_sb = persist.tile([P, B, NJ * C1], F32)
            nc.sync.dma_start(out=xa_sb[:, 0, :], in_=xa_d[0])
            nc.sync.dma_start(out=dT_sb[:, 4:8, :], in_=dTview[:, 4:8, :])
            nc.scalar.dma_start(out=xa_sb[:, 1, :], in_=xa_d[1])

            ebst = {}

            eb1_sb = persist.tile([P, P], F32)

            def v_steps():
                # both batches in one [32, 256] chain
                ps_v = psT.tile([H, B * P], F32, tag="ps")
                nc.tensor.matmul(ps_v[:], wm, xoT_sb[:])
                y = work.tile([H, B * P], F32, tag="y")
                nc.vector.tensor_scalar(
                    out=y[:], in0=ps_v[:], scalar1=vcol,
                    scalar2=None, op0=Alu.add,
                )
                t1 = work.tile([H, B * P], F32, tag="t1")
                nc.vector.tensor_scalar(
                    out=t1[:], in0=ps_v[:], scalar1=vcol, scalar2=ALPHA,
                    op0=Alu.add, op1=Alu.mult,
                )
                t2 = work.tile([H, B * P], F32, tag="t2")
                nc.vector.tensor_tensor(out=t2[:], in0=y[:], in1=t1[:], op=Alu.max)
                eT = work.tile([H, B * P], F32, tag="eT")
                nc.scalar.activation(eT[:], t2[:], Act.Exp)
                ebst["e"] = eT
                # b=1 broadcast via replicate DMA (latency hidden by b0 work)
                src = eT[:, P : 2 * P]
                rep = bass.AP(
                    tensor=src.tensor, offset=src.offset,
                    ap=[list(src.ap[0]), [0, 4], list(src.ap[1])],
                )
                nc.sync.dma_start(out=eb1_sb[:], in_=rep)
                ebst[1] = eb1_sb

            def eb_mm():
                # b=0 broadcast eb[q, I] = e[q//4, I] via indicator matmul
                ps_eb = psE.tile([P, P], F32, tag="eb", name="eb0")
                nc.tensor.matmul(ps_eb[:], ind, ebst["e"][:, 0:P])
                ebst[0] = ps_eb

            aggst = {}

            def agg_step(b, tk):
                if tk == 0:
                    aggst[b] = (
                        psA.tile([C1, P], F32, tag="pse", name=f"pse{b}"),
                        psA.tile([C1, P], F32, tag="pso", name=f"pso{b}"),
                    )
                wtile = work.tile([P, P], F32, tag="wtile")
                nc.vector.scalar_tensor_tensor(
                    out=wtile[:], in0=dT_sb[:, tk, :], scalar=THR,
                    in1=ebst[b][:], op0=Alu.is_lt, op1=Alu.mult,
                )
                ps_g = aggst[b][tk % 2]
                nc.tensor.matmul(
                    ps_g[:],
                    xa_sb[:, b, C1 * tk : C1 * tk + C1],
                    wtile[:],
                    start=(tk < 2),
                    stop=(tk >= NJ - 2),
                    skip_group_check=True,
                )

            def finalize(b):
                ps_even, ps_odd = aggst[b]
                gtmp = work.tile([C1, P], F32, tag="gtmp")
                nc.vector.tensor_copy(gtmp[:], ps_odd[:])
                gt = work.tile([C1, P], F32, tag="gt")
                nc.vector.tensor_add(out=gt[:], in0=ps_even[:], in1=gtmp[:])
                ps_f = psT.tile([P, H1], F32, tag="ps", name=f"psf{b}")
                nc.tensor.matmul(ps_f[:], gt[:], wfin)
                ot = work.tile([P, H1], F32, tag="ot", name=f"ot{b}")
                nc.vector.tensor_copy(ot[:], ps_f[:])
                eng = nc.sync if b == 0 else nc.scalar
                eng.dma_start(out=out_d[b], in_=ot[:])

            # ---- interleaved emission ----
            v_steps()
            eb_mm()
            agg_step(0, 0)
            agg_step(0, 1)
            agg_step(0, 2)
            agg_step(0, 3)
            agg_step(0, 4)
            agg_step(0, 5)
            agg_step(0, 6)
            agg_step(0, 7)
            agg_step(1, 0)
            agg_step(1, 1)
            finalize(0)
            for tk in range(2, NJ):
                agg_step(1, tk)
            finalize(1)

    nc.finalize()
    return nc


def kernel(x, dist_mat, proj_w, proj_b, a_w, trace=False):
    global LAST_RESULT
    from concourse.bass_utils import run_bass_kernel_spmd

    x = np.ascontiguousarray(np.asarray(x, dtype=np.float32))
    dist_mat = np.asarray(dist_mat, dtype=np.float32)
    proj_w = np.asarray(proj_w, dtype=np.float32)
    proj_b = np.asarray(proj_b, dtype=np.float32).reshape(H)
    a_w = np.asarray(a_w, dtype=np.float32).reshape(2 * H)

    if "nc" not in _CACHE:
        _CACHE["nc"] = _build()
    nc = _CACHE["nc"]

    # ---- host-side constant folding (all tiny) ----
    a1, a2 = a_w[:H], a_w[H:]
    s1 = np.float32(a1.sum(dtype=np.float32))
    m32 = s1 * np.eye(H, dtype=np.float32) + a2[:, None]  # v = m32.T @ hT
    wta = proj_w.T.astype(np.float32)                     # [C, H]
    wm = np.ascontiguousarray(wta @ m32)                  # fold h->v projection
    vcol = np.ascontiguousarray((m32.T @ proj_b).reshape(H, 1))
    # final projection [x | 1] -> [h | 1]: bias row, ones column slot
    wfin = np.zeros((C1, H1), np.float32)
    wfin[:C, :H] = wta
    wfin[C, :H] = proj_b
    wfin[C, H] = 1.0
    # eb indicator: ind[k, q] = 1 iff q//4 == k
    ind = np.zeros((H, P), np.float32)
    for k in range(H):
        ind[k, 4 * k : 4 * k + 4] = 1.0
    # pack all the small constants into one block -> a single DMA
    cblk0 = np.zeros((P, 256), np.float32)
    cblk0[0:C, 0:H] = wm
    cblk0[0:H, H : H + 1] = vcol
    cblk0[0:H, 64:192] = ind
    cblk0[0:C1, 192 : 192 + H1] = wfin

    dist_fixed = dist_mat.copy()
    np.fill_diagonal(dist_fixed, 0.0)  # adj diagonal forced to 1
    distT = np.ascontiguousarray(dist_fixed.T)

    # token J = 8*q + tk; row q of xa[b] holds tokens 8q..8q+7, each
    # augmented with a trailing 1.0 (for the softmax denominator)
    xa = np.ones((B, N, C1), np.float32)
    xa[:, :, :C] = x
    xa = xa.reshape(B, P, NJ * C1)

    in_maps = []
    for c in range(NCORES):
        sl = slice(c * P, (c + 1) * P)
        in_maps.append(
            {
                "xa": xa,
                "xoT": np.concatenate([x[0, sl, :].T, x[1, sl, :].T], axis=1),
                "distT": np.ascontiguousarray(distT[:, sl]),
                "cblk": cblk0,
            }
        )

    res = run_bass_kernel_spmd(nc, in_maps, core_ids=list(range(NCORES)), trace=trace)
    LAST_RESULT = res
    full = np.concatenate([res.results[c]["out"] for c in range(NCORES)], axis=1)
    return np.ascontiguousarray(full[:, :, :H] / full[:, :, H : H + 1])


# revision 38
# speedup vs baseline: 1.1141x; 1.0778x over previous
"""GAT message-passing kernel for 8 Trainium2 NeuronCores.

Key algebraic property of the reference (faithful torch repeat_interleave
replication): with h = x @ proj_w.T + proj_b  [B, N, H],
    first[b, I, J, c]  = h[b, I, (J*H+c) // N] = h[b, I, J // (N//H)]
    second[b, I, J, c] = h[b, I, c]
so the pre-mask score collapses to
    scores[b, I, J] = leaky_relu(S1 * h[b, I, J//32] + d[b, I])
with S1 = sum(a_w[0, :H]) and d = h @ a_w[0, H:].  Each row of scores has
only H=32 distinct values (one per 32-column block of J).  Softmax+matmul
then reduce to a masked weighted aggregation that never materializes any
[N, N] tensor in HBM:
    W[b, I, J] = adj[I, J] * exp(leaky(v))[b, I, J//32]
    out[b, I, :] = (W @ h[b]) / rowsum(W)

Sharding: rows I are split 128-per-core across 8 cores (both batches on
every core).  dist_mat rows are sharded; x and the tiny weights are
replicated.

Two structural tricks shrink device work to a handful of PE ops:

1. J-side tiling J = 8*q + tk (q = partition, tk = tile 0..7): x loads
   become fully contiguous and k(J) = J//32 = q//4 for every tile, so the
   score-broadcast tile eb[q, I] = e[q//4, I] is shared by all 8 J-tiles
   (built once per batch: an indicator matmul for batch 0, a replicate
   DMA for batch 1 whose latency hides under batch-0 work).

2. Aggregation associativity: out = (W @ x_aug) @ wt_aug, with
   x_aug = [x | 1].  The kernel accumulates GT[cc, I] = sum_J
   x_aug[J, cc] * W.T[J, I] with PSUM-accumulated matmuls whose lhsT is
   the *natural-layout* x_aug tile — h is never materialized and x is
   never transposed.  GT[64, :] = Z (softmax denominator).  One final
   [65, 33] matmul per batch applies wt_aug (bias row included; its ones
   column carries Z into column 32 of the output); the divide by Z
   happens on host on the [B, N, 33] result.

adjT tiles come from PE transposes of stride-8 column slices of the
core's dist rows (diagonal pre-zeroed on host => forced diag-1), compared
against the threshold on DVE.  W.T tiles are adjT * eb elementwise.  The
serial v-path (scores, one fused [32, 2*128] chain for both batches) is
interleaved with the adj transposes so PE never idles, and GT
accumulation alternates two PSUM banks to pipeline the fp32 passes.
"""

import sys

sys.path.insert(0, "/opt/trn_rl_repo")

import numpy as np

B, N, C, H = 2, 1024, 64, 32
P = 128                 # rows per core / partition tile
NCORES = 8
NJ = N // P             # 8 J-tiles of 128
THR = 200000.0
ALPHA = 0.01
H1 = H + 1              # 33: h channels + ones column
C1 = C + 1              # 65: x channels + ones column

_CACHE = {}
LAST_RESULT = None


def _build():
    import concourse.bacc as bacc
    import concourse.bass as bass
    import concourse.tile as tile
    from concourse import masks, mybir

    F32 = mybir.dt.float32
    Alu = mybir.AluOpType
    Act = mybir.ActivationFunctionType

    nc = bacc.Bacc("TRN2", target_bir_lowering=False)

    xa_d = nc.dram_tensor("xa", (B, P, NJ * C1), F32, kind="ExternalInput")
    xoT_d = nc.dram_tensor("xoT", (C, B * P), F32, kind="ExternalInput")
    distT_d = nc.dram_tensor("distT", (N, P), F32, kind="ExternalInput")
    # packed consts: cols 0:32 wm (rows 0:64), 32:33 vcol (rows 0:32),
    # 64:192 ind (rows 0:32), 192:225 wfin (rows 0:65)
    cblk_d = nc.dram_tensor("cblk", (P, 256), F32, kind="ExternalInput")
    # un-normalized output + Z column; the divide happens on host
    out_d = nc.dram_tensor("out", (B, P, H1), F32, kind="ExternalOutput")

    with tile.TileContext(nc) as tc:
        with (
            tc.tile_pool(name="const", bufs=1) as const,
            tc.tile_pool(name="persist", bufs=1) as persist,
            tc.tile_pool(name="work", bufs=3) as work,
            tc.tile_pool(name="psT", bufs=2, space="PSUM") as psT,
            tc.tile_pool(name="psE", bufs=2, space="PSUM") as psE,
            tc.tile_pool(name="psA", bufs=2, space="PSUM") as psA,
        ):
            # ---- input DMAs: all large loads are fully contiguous ----
            cblk = const.tile([P, 256], F32)
            nc.scalar.dma_start(out=cblk[:], in_=cblk_d[:])
            wm = cblk[0:C, 0:H]
            vcol = cblk[0:H, H : H + 1]
            ind = cblk[0:H, 64:192]
            wfin = cblk[0:C1, 192 : 192 + H1]
            xoT_sb = const.tile([C, B * P], F32)
            nc.sync.dma_start(out=xoT_sb[:], in_=xoT_d[:])
            # distT rows J = 8q + tk: partition q reads 4KB contiguous
            dT_sb = persist.tile([P, NJ, P], F32)
            dTview = distT_d[:].rearrange("(q t) i -> q t i", t=NJ)
            nc.sync.dma_start(out=dT_sb[:, 0:4, :], in_=dTview[:, 0:4, :])
            xa_sb = persist.tile([P, B, NJ * C1], F32)
            nc.scalar.dma_start(out=xa_sb[:, 0, :], in_=xa_d[0])
            nc.sync.dma_start(out=dT_sb[:, 4:8, :], in_=dTview[:, 4:8, :])
            nc.scalar.dma_start(out=xa_sb[:, 1, :], in_=xa_d[1])

            ebst = {}

            eb1_sb = persist.tile([P, P], F32)

            def v_steps():
                # both batches in one [32, 256] chain
                ps_v = psT.tile([H, B * P], F32, tag="ps")
                nc.tensor.matmul(ps_v[:], wm, xoT_sb[:])
                y = work.tile([H, B * P], F32, tag="y")
                nc.vector.tensor_scalar(
                    out=y[:], in0=ps_v[:], scalar1=vcol,
                    scalar2=None, op0=Alu.add,
                )
                t1 = work.tile([H, B * P], F32, tag="t1")
                nc.vector.tensor_scalar(
                    out=t1[:], in0=ps_v[:], scalar1=vcol, scalar2=ALPHA,
                    op0=Alu.add, op1=Alu.mult,
                )
                t2 = work.tile([H, B * P], F32, tag="t2")
                nc.vector.tensor_tensor(out=t2[:], in0=y[:], in1=t1[:], op=Alu.max)
                eT = work.tile([H, B * P], F32, tag="eT")
                nc.scalar.activation(eT[:], t2[:], Act.Exp)
                ebst["e"] = eT
                # b=1 broadcast via replicate DMA (latency hidden by b0 work)
                src = eT[:, P : 2 * P]
                rep = bass.AP(
                    tensor=src.tensor, offset=src.offset,
                    ap=[list(src.ap[0]), [0, 4], list(src.ap[1])],
                )
                nc.sync.dma_start(out=eb1_sb[:], in_=rep)
                ebst[1] = eb1_sb

            def eb_mm():
                # b=0 broadcast eb[q, I] = e[q//4, I] via indicator matmul
                ps_eb = psE.tile([P, P], F32, tag="eb", name="eb0")
                nc.tensor.matmul(ps_eb[:], ind, ebst["e"][:, 0:P])
                ebst[0] = ps_eb

            aggst = {}

            def agg_step(b, tk):
                if tk == 0:
                    aggst[b] = (
                        psA.tile([C1, P], F32, tag="pse", name=f"pse{b}"),
                        psA.tile([C1, P], F32, tag="pso", name=f"pso{b}"),
                    )
                wtile = work.tile([P, P], F32, tag="wtile")
                nc.vector.scalar_tensor_tensor(
                    out=wtile[:], in0=dT_sb[:, tk, :], scalar=THR,
                    in1=ebst[b][:], op0=Alu.is_lt, op1=Alu.mult,
                )
                ps_g = aggst[b][tk % 2]
                nc.tensor.matmul(
                    ps_g[:],
                    xa_sb[:, b, C1 * tk : C1 * tk + C1],
                    wtile[:],
                    start=(tk < 2),
                    stop=(tk >= NJ - 2),
                    skip_group_check=True,
                )

            def finalize(b):
                ps_even, ps_odd = aggst[b]
                gtmp = work.tile([C1, P], F32, tag="gtmp")
                nc.vector.tensor_copy(gtmp[:], ps_odd[:])
                gt = work.tile([C1, P], F32, tag="gt")
                nc.vector.tensor_add(out=gt[:], in0=ps_even[:], in1=gtmp[:])
                ps_f = psT.tile([P, H1], F32, tag="ps", name=f"psf{b}")
                nc.tensor.matmul(ps_f[:], gt[:], wfin)
                ot = work.tile([P, H1], F32, tag="ot", name=f"ot{b}")
                nc.vector.tensor_copy(ot[:], ps_f[:])
                eng = nc.sync if b == 0 else nc.scalar
                eng.dma_start(out=out_d[b], in_=ot[:])

            # ---- interleaved emission ----
            v_steps()
            eb_mm()
            agg_step(0, 0)
            agg_step(0, 1)
            agg_step(0, 2)
            agg_step(0, 3)
            agg_step(0, 4)
            agg_step(0, 5)
            agg_step(0, 6)
            agg_step(0, 7)
            agg_step(1, 0)
            agg_step(1, 1)
            finalize(0)
            for tk in range(2, NJ):
                agg_step(1, tk)
            finalize(1)

    nc.finalize()
    return nc


def kernel(x, dist_mat, proj_w, proj_b, a_w, trace=False):
    global LAST_RESULT
    from concourse.bass_utils import run_bass_kernel_spmd

    x = np.ascontiguousarray(np.asarray(x, dtype=np.float32))
    dist_mat = np.asarray(dist_mat, dtype=np.float32)
    proj_w = np.asarray(proj_w, dtype=np.float32)
    proj_b = np.asarray(proj_b, dtype=np.float32).reshape(H)
    a_w = np.asarray(a_w, dtype=np.float32).reshape(2 * H)

    if "nc" not in _CACHE:
        _CACHE["nc"] = _build()
    nc = _CACHE["nc"]

    # ---- host-side constant folding (all tiny) ----
    a1, a2 = a_w[:H], a_w[H:]
    s1 = np.float32(a1.sum(dtype=np.float32))
    m32 = s1 * np.eye(H, dtype=np.float32) + a2[:, None]  # v = m32.T @ hT
    wta = proj_w.T.astype(np.float32)                     # [C, H]
    wm = np.ascontiguousarray(wta @ m32)                  # fold h->v projection
    vcol = np.ascontiguousarray((m32.T @ proj_b).reshape(H, 1))
    # final projection [x | 1] -> [h | 1]: bias row, ones column slot
    wfin = np.zeros((C1, H1), np.float32)
    wfin[:C, :H] = wta
    wfin[C, :H] = proj_b
    wfin[C, H] = 1.0
    # eb indicator: ind[k, q] = 1 iff q//4 == k
    ind = np.zeros((H, P), np.float32)
    for k in range(H):
        ind[k, 4 * k : 4 * k + 4] = 1.0
    # pack all the small constants into one block -> a single DMA
    cblk0 = np.zeros((P, 256), np.float32)
    cblk0[0:C, 0:H] = wm
    cblk0[0:H, H : H + 1] = vcol
    cblk0[0:H, 64:192] = ind
    cblk0[0:C1, 192 : 192 + H1] = wfin

    dist_fixed = dist_mat.copy()
    np.fill_diagonal(dist_fixed, 0.0)  # adj diagonal forced to 1
    distT = np.ascontiguousarray(dist_fixed.T)

    # token J = 8*q + tk; row q of xa[b] holds tokens 8q..8q+7, each
    # augmented with a trailing 1.0 (for the softmax denominator)
    xa = np.ones((B, N, C1), np.float32)
    xa[:, :, :C] = x
    xa = xa.reshape(B, P, NJ * C1)

    in_maps = []
    for c in range(NCORES):
        sl = slice(c * P, (c + 1) * P)
        in_maps.append(
            {
                "xa": xa,
                "xoT": np.concatenate([x[0, sl, :].T, x[1, sl, :].T], axis=1),
                "distT": np.ascontiguousarray(distT[:, sl]),
                "cblk": cblk0,
            }
        )

    res = run_bass_kernel_spmd(nc, in_maps, core_ids=list(range(NCORES)), trace=trace)
    LAST_RESULT = res
    full = np.concatenate([res.results[c]["out"] for c in range(NCORES)], axis=1)
    return np.ascontiguousarray(full[:, :, :H] / full[:, :, H : H + 1])


# revision 39
# speedup vs baseline: 1.1328x; 1.0168x over previous
"""GAT message-passing kernel for 8 Trainium2 NeuronCores.

Key algebraic property of the reference (faithful torch repeat_interleave
replication): with h = x @ proj_w.T + proj_b  [B, N, H],
    first[b, I, J, c]  = h[b, I, (J*H+c) // N] = h[b, I, J // (N//H)]
    second[b, I, J, c] = h[b, I, c]
so the pre-mask score collapses to
    scores[b, I, J] = leaky_relu(S1 * h[b, I, J//32] + d[b, I])
with S1 = sum(a_w[0, :H]) and d = h @ a_w[0, H:].  Each row of scores has
only H=32 distinct values (one per 32-column block of J).  Softmax+matmul
then reduce to a masked weighted aggregation that never materializes any
[N, N] tensor in HBM:
    W[b, I, J] = adj[I, J] * exp(leaky(v))[b, I, J//32]
    out[b, I, :] = (W @ h[b]) / rowsum(W)

Sharding: rows I are split 128-per-core across 8 cores (both batches on
every core).  dist_mat rows are sharded; x and the tiny weights are
replicated.

Two structural tricks shrink device work to a handful of PE ops:

1. J-side tiling J = 8*q + tk (q = partition, tk = tile 0..7): x loads
   become fully contiguous and k(J) = J//32 = q//4 for every tile, so the
   score-broadcast tile eb[q, I] = e[q//4, I] is shared by all 8 J-tiles
   (built once per batch: an indicator matmul for batch 0, a replicate
   DMA for batch 1 whose latency hides under batch-0 work).

2. Aggregation associativity: out = (W @ x_aug) @ wt_aug, with
   x_aug = [x | 1].  The kernel accumulates GT[cc, I] = sum_J
   x_aug[J, cc] * W.T[J, I] with PSUM-accumulated matmuls whose lhsT is
   the *natural-layout* x_aug tile — h is never materialized and x is
   never transposed.  GT[64, :] = Z (softmax denominator).  One final
   [65, 33] matmul per batch applies wt_aug (bias row included; its ones
   column carries Z into column 32 of the output); the divide by Z
   happens on host on the [B, N, 33] result.

The mask side needs dist.T (diagonal pre-zeroed on host => forced
diag-1); the host supplies the transposed slice so each SBUF partition
reads 4KB contiguous, and the threshold compare is fused into the W.T
tile build: wtile = (distT < thr) * eb in a single scalar_tensor_tensor
DVE op — the kernel runs zero PE transposes.  The first-needed inputs
(xoT | cblk) load in parallel on the two HWDGE rings, the serial v-path
(one fused [32, 2*128] chain for both batches) overlaps the loads, and
GT accumulation alternates two PSUM banks to pipeline the fp32 passes.
"""

import sys

sys.path.insert(0, "/opt/trn_rl_repo")

import numpy as np

B, N, C, H = 2, 1024, 64, 32
P = 128                 # rows per core / partition tile
NCORES = 8
NJ = N // P             # 8 J-tiles of 128
THR = 200000.0
ALPHA = 0.01
H1 = H + 1              # 33: h channels + ones column
C1 = C + 1              # 65: x channels + ones column

_CACHE = {}
LAST_RESULT = None


def _build():
    import concourse.bacc as bacc
    import concourse.bass as bass
    import concourse.tile as tile
    from concourse import masks, mybir

    F32 = mybir.dt.float32
    Alu = mybir.AluOpType
    Act = mybir.ActivationFunctionType

    nc = bacc.Bacc("TRN2", target_bir_lowering=False)

    xa_d = nc.dram_tensor("xa", (B, P, NJ * C1), F32, kind="ExternalInput")
    xoT_d = nc.dram_tensor("xoT", (C, B * P), F32, kind="ExternalInput")
    distT_d = nc.dram_tensor("distT", (N, P), F32, kind="ExternalInput")
    # packed consts: cols 0:32 wm (rows 0:64), 32:33 vcol (rows 0:32),
    # 64:192 ind (rows 0:32), 192:225 wfin (rows 0:65)
    cblk_d = nc.dram_tensor("cblk", (P, 256), F32, kind="ExternalInput")
    # un-normalized output + Z column; the divide happens on host
    out_d = nc.dram_tensor("out", (B, P, H1), F32, kind="ExternalOutput")

    with tile.TileContext(nc) as tc:
        with (
            tc.tile_pool(name="const", bufs=1) as const,
            tc.tile_pool(name="persist", bufs=1) as persist,
            tc.tile_pool(name="work", bufs=3) as work,
            tc.tile_pool(name="psT", bufs=2, space="PSUM") as psT,
            tc.tile_pool(name="psE", bufs=2, space="PSUM") as psE,
            tc.tile_pool(name="psA", bufs=2, space="PSUM") as psA,
        ):
            # ---- input DMAs: all large loads are fully contiguous ----
            cblk = const.tile([P, 256], F32)
            nc.scalar.dma_start(out=cblk[:], in_=cblk_d[:])
            wm = cblk[0:C, 0:H]
            vcol = cblk[0:H, H : H + 1]
            ind = cblk[0:H, 64:192]
            wfin = cblk[0:C1, 192 : 192 + H1]
            xoT_sb = const.tile([C, B * P], F32)
            nc.sync.dma_start(out=xoT_sb[:], in_=xoT_d[:])
            # distT rows J = 8q + tk: partition q reads 4KB contiguous
            dT_sb = persist.tile([P, NJ, P], F32)
            dTview = distT_d[:].rearrange("(q t) i -> q t i", t=NJ)
            nc.sync.dma_start(out=dT_sb[:, 0:4, :], in_=dTview[:, 0:4, :])
            xa_sb = persist.tile([P, B, NJ * C1], F32)
            nc.scalar.dma_start(out=xa_sb[:, 0, :], in_=xa_d[0])
            nc.sync.dma_start(out=dT_sb[:, 4:8, :], in_=dTview[:, 4:8, :])
            nc.scalar.dma_start(out=xa_sb[:, 1, :], in_=xa_d[1])

            ebst = {}

            eb1_sb = persist.tile([P, P], F32)

            def v_steps():
                # both batches in one [32, 256] chain
                ps_v = psT.tile([H, B * P], F32, tag="ps")
                nc.tensor.matmul(ps_v[:], wm, xoT_sb[:])
                y = work.tile([H, B * P], F32, tag="y")
                nc.vector.tensor_scalar(
                    out=y[:], in0=ps_v[:], scalar1=vcol,
                    scalar2=None, op0=Alu.add,
                )
                t1 = work.tile([H, B * P], F32, tag="t1")
                nc.vector.tensor_scalar(
                    out=t1[:], in0=ps_v[:], scalar1=vcol, scalar2=ALPHA,
                    op0=Alu.add, op1=Alu.mult,
                )
                t2 = work.tile([H, B * P], F32, tag="t2")
                nc.vector.tensor_tensor(out=t2[:], in0=y[:], in1=t1[:], op=Alu.max)
                eT = work.tile([H, B * P], F32, tag="eT")
                nc.scalar.activation(eT[:], t2[:], Act.Exp)
                ebst["e"] = eT
                # b=1 broadcast via replicate DMA (latency hidden by b0 work)
                src = eT[:, P : 2 * P]
                rep = bass.AP(
                    tensor=src.tensor, offset=src.offset,
                    ap=[list(src.ap[0]), [0, 4], list(src.ap[1])],
                )
                nc.sync.dma_start(out=eb1_sb[:], in_=rep)
                ebst[1] = eb1_sb

            def eb_mm():
                # b=0 broadcast eb[q, I] = e[q//4, I] via indicator matmul
                ps_eb = psE.tile([P, P], F32, tag="eb", name="eb0")
                nc.tensor.matmul(ps_eb[:], ind, ebst["e"][:, 0:P])
                ebst[0] = ps_eb

            aggst = {}

            def agg_step(b, tk):
                if tk == 0:
                    aggst[b] = (
                        psA.tile([C1, P], F32, tag="pse", name=f"pse{b}"),
                        psA.tile([C1, P], F32, tag="pso", name=f"pso{b}"),
                    )
                wtile = work.tile([P, P], F32, tag="wtile")
                nc.vector.scalar_tensor_tensor(
                    out=wtile[:], in0=dT_sb[:, tk, :], scalar=THR,
                    in1=ebst[b][:], op0=Alu.is_lt, op1=Alu.mult,
                )
                ps_g = aggst[b][tk % 2]
                nc.tensor.matmul(
                    ps_g[:],
                    xa_sb[:, b, C1 * tk : C1 * tk + C1],
                    wtile[:],
                    start=(tk < 2),
                    stop=(tk >= NJ - 2),
                    skip_group_check=True,
                )

            def finalize(b):
                ps_even, ps_odd = aggst[b]
                gtmp = work.tile([C1, P], F32, tag="gtmp")
                nc.vector.tensor_copy(gtmp[:], ps_odd[:])
                gt = work.tile([C1, P], F32, tag="gt")
                nc.vector.tensor_add(out=gt[:], in0=ps_even[:], in1=gtmp[:])
                ps_f = psT.tile([P, H1], F32, tag="ps", name=f"psf{b}")
                nc.tensor.matmul(ps_f[:], gt[:], wfin)
                ot = work.tile([P, H1], F32, tag="ot", name=f"ot{b}")
                nc.vector.tensor_copy(ot[:], ps_f[:])
                eng = nc.sync if b == 0 else nc.scalar
                eng.dma_start(out=out_d[b], in_=ot[:])

            # ---- interleaved emission ----
            v_steps()
            eb_mm()
            agg_step(0, 0)
            agg_step(0, 1)
            agg_step(0, 2)
            agg_step(0, 3)
            agg_step(0, 4)
            agg_step(0, 5)
            agg_step(0, 6)
            agg_step(0, 7)
            agg_step(1, 0)
            agg_step(1, 1)
            finalize(0)
            for tk in range(2, NJ):
                agg_step(1, tk)
            finalize(1)

    nc.finalize()
    return nc


def kernel(x, dist_mat, proj_w, proj_b, a_w, trace=False):
    global LAST_RESULT
    from concourse.bass_utils import run_bass_kernel_spmd

    x = np.ascontiguousarray(np.asarray(x, dtype=np.float32))
    dist_mat = np.asarray(dist_mat, dtype=np.float32)
    proj_w = np.asarray(proj_w, dtype=np.float32)
    proj_b = np.asarray(proj_b, dtype=np.float32).reshape(H)
    a_w = np.asarray(a_w, dtype=np.float32).reshape(2 * H)

    if "nc" not in _CACHE:
        _CACHE["nc"] = _build()
    nc = _CACHE["nc"]

    # ---- host-side constant folding (all tiny) ----
    a1, a2 = a_w[:H], a_w[H:]
    s1 = np.float32(a1.sum(dtype=np.float32))
    m32 = s1 * np.eye(H, dtype=np.float32) + a2[:, None]  # v = m32.T @ hT
    wta = proj_w.T.astype(np.float32)                     # [C, H]
    wm = np.ascontiguousarray(wta @ m32)                  # fold h->v projection
    vcol = np.ascontiguousarray((m32.T @ proj_b).reshape(H, 1))
    # final projection [x | 1] -> [h | 1]: bias row, ones column slot
    wfin = np.zeros((C1, H1), np.float32)
    wfin[:C, :H] = wta
    wfin[C, :H] = proj_b
    wfin[C, H] = 1.0
    # eb indicator: ind[k, q] = 1 iff q//4 == k
    ind = np.zeros((H, P), np.float32)
    for k in range(H):
        ind[k, 4 * k : 4 * k + 4] = 1.0
    # pack all the small constants into one block -> a single DMA
    cblk0 = np.zeros((P, 256), np.float32)
    cblk0[0:C, 0:H] = wm
    cblk0[0:H, H : H + 1] = vcol
    cblk0[0:H, 64:192] = ind
    cblk0[0:C1, 192 : 192 + H1] = wfin

    dist_fixed = dist_mat.copy()
    np.fill_diagonal(dist_fixed, 0.0)  # adj diagonal forced to 1
    distT = np.ascontiguousarray(dist_fixed.T)

    # token J = 8*q + tk; row q of xa[b] holds tokens 8q..8q+7, each
    # augmented with a trailing 1.0 (for the softmax denominator)
    xa = np.ones((B, N, C1), np.float32)
    xa[:, :, :C] = x
    xa = xa.reshape(B, P, NJ * C1)

    in_maps = []
    for c in range(NCORES):
        sl = slice(c * P, (c + 1) * P)
        in_maps.append(
            {
                "xa": xa,
                "xoT": np.concatenate([x[0, sl, :].T, x[1, sl, :].T], axis=1),
                "distT": np.ascontiguousarray(distT[:, sl]),
                "cblk": cblk0,
            }
        )

    res = run_bass_kernel_spmd(nc, in_maps, core_ids=list(range(NCORES)), trace=trace)
    LAST_RESULT = res
    full = np.concatenate([res.results[c]["out"] for c in range(NCORES)], axis=1)
    return np.ascontiguousarray(full[:, :, :H] / full[:, :, H : H + 1])


# revision 40
# speedup vs baseline: 1.1743x; 1.0367x over previous
"""GAT message-passing kernel for 8 Trainium2 NeuronCores.

Key algebraic property of the reference (faithful torch repeat_interleave
replication): with h = x @ proj_w.T + proj_b  [B, N, H],
    first[b, I, J, c]  = h[b, I, (J*H+c) // N] = h[b, I, J // (N//H)]
    second[b, I, J, c] = h[b, I, c]
so the pre-mask score collapses to
    scores[b, I, J] = leaky_relu(S1 * h[b, I, J//32] + d[b, I])
with S1 = sum(a_w[0, :H]) and d = h @ a_w[0, H:].  Each row of scores has
only H=32 distinct values (one per 32-column block of J).  Softmax+matmul
then reduce to a masked weighted aggregation that never materializes any
[N, N] tensor in HBM:
    W[b, I, J] = adj[I, J] * exp(leaky(v))[b, I, J//32]
    out[b, I, :] = (W @ h[b]) / rowsum(W)

Sharding: rows I are split 128-per-core across 8 cores (both batches on
every core).  dist_mat rows are sharded; x and the tiny weights are
replicated.

Two structural tricks shrink device work to a handful of PE ops:

1. J-side tiling J = 8*q + tk (q = partition, tk = tile 0..7): x loads
   become fully contiguous and k(J) = J//32 = q//4 for every tile, so the
   score-broadcast tile eb[q, I] = e[q//4, I] is shared by all 8 J-tiles
   (built once per batch: an indicator matmul for batch 0, a replicate
   DMA for batch 1 whose latency hides under batch-0 work).

2. Aggregation associativity: out = (W @ x_aug) @ wt_aug, with
   x_aug = [x | 1].  The kernel accumulates GT[cc, I] = sum_J
   x_aug[J, cc] * W.T[J, I] with PSUM-accumulated matmuls whose lhsT is
   the *natural-layout* x_aug tile — h is never materialized and x is
   never transposed.  GT[64, :] = Z (softmax denominator).  One final
   [65, 33] matmul per batch applies wt_aug (bias row included; its ones
   column carries Z into column 32 of the output); the divide by Z
   happens on host on the [B, N, 33] result.

The mask side needs dist.T (diagonal pre-zeroed on host => forced
diag-1); the host supplies the transposed slice so each SBUF partition
reads 4KB contiguous, and the threshold compare is fused into the W.T
tile build: wtile = (distT < thr) * eb in a single scalar_tensor_tensor
DVE op — the kernel runs zero PE transposes.  The first-needed inputs
(xoT | cblk) load in parallel on the two HWDGE rings, the serial v-path
(one fused [32, 2*128] chain for both batches) overlaps the loads, and
GT accumulation alternates two PSUM banks to pipeline the fp32 passes.
"""

import sys

sys.path.insert(0, "/opt/trn_rl_repo")

import numpy as np

B, N, C, H = 2, 1024, 64, 32
P = 128                 # rows per core / partition tile
NCORES = 8
NJ = N // P             # 8 J-tiles of 128
THR = 200000.0
ALPHA = 0.01
H1 = H + 1              # 33: h channels + ones column
C1 = C + 1              # 65: x channels + ones column

_CACHE = {}
LAST_RESULT = None


def _build():
    import concourse.bacc as bacc
    import concourse.bass as bass
    import concourse.tile as tile
    from concourse import masks, mybir

    F32 = mybir.dt.float32
    Alu = mybir.AluOpType
    Act = mybir.ActivationFunctionType

    nc = bacc.Bacc("TRN2", target_bir_lowering=False)

    xa_d = nc.dram_tensor("xa", (B, P, NJ * C1), F32, kind="ExternalInput")
    xoT_d = nc.dram_tensor("xoT", (C, B * P), F32, kind="ExternalInput")
    distT_d = nc.dram_tensor("distT", (N, P), F32, kind="ExternalInput")
    # packed consts: cols 0:32 wm (rows 0:64), 32:33 vcol (rows 0:32),
    # 64:192 ind (rows 0:32), 192:225 wfin (rows 0:65)
    cblk_d = nc.dram_tensor("cblk", (P, 256), F32, kind="ExternalInput")
    # un-normalized output + Z column; the divide happens on host
    out_d = nc.dram_tensor("out", (B, P, H1), F32, kind="ExternalOutput")

    with tile.TileContext(nc) as tc:
        with (
            tc.tile_pool(name="const", bufs=1) as const,
            tc.tile_pool(name="persist", bufs=1) as persist,
            tc.tile_pool(name="work", bufs=3) as work,
            tc.tile_pool(name="psT", bufs=2, space="PSUM") as psT,
            tc.tile_pool(name="psE", bufs=2, space="PSUM") as psE,
            tc.tile_pool(name="psA", bufs=2, space="PSUM") as psA,
        ):
            # ---- input DMAs: all large loads are fully contiguous ----
            cblk = const.tile([P, 256], F32)
            nc.scalar.dma_start(out=cblk[:], in_=cblk_d[:])
            wm = cblk[0:C, 0:H]
            vcol = cblk[0:H, H : H + 1]
            ind = cblk[0:H, 64:192]
            wfin = cblk[0:C1, 192 : 192 + H1]
            xoT_sb = const.tile([C, B * P], F32)
            nc.sync.dma_start(out=xoT_sb[:], in_=xoT_d[:])
            # distT rows J = 8q + tk: partition q reads 4KB contiguous
            dT_sb = persist.tile([P, NJ, P], F32)
            dTview = distT_d[:].rearrange("(q t) i -> q t i", t=NJ)
            nc.sync.dma_start(out=dT_sb[:, 0:4, :], in_=dTview[:, 0:4, :])
            xa_sb = persist.tile([P, B, NJ * C1], F32)
            nc.scalar.dma_start(out=xa_sb[:, 0, :], in_=xa_d[0])
            nc.sync.dma_start(out=dT_sb[:, 4:8, :], in_=dTview[:, 4:8, :])
            nc.scalar.dma_start(out=xa_sb[:, 1, :], in_=xa_d[1])

            ebst = {}

            eb1_sb = persist.tile([P, P], F32)

            def v_steps():
                # both batches in one [32, 256] chain
                ps_v = psT.tile([H, B * P], F32, tag="ps")
                nc.tensor.matmul(ps_v[:], wm, xoT_sb[:])
                t1 = work.tile([H, B * P], F32, tag="t1")
                nc.vector.tensor_scalar(
                    out=t1[:], in0=ps_v[:], scalar1=vcol, scalar2=ALPHA,
                    op0=Alu.add, op1=Alu.mult,
                )
                t2 = work.tile([H, B * P], F32, tag="t2")
                nc.vector.scalar_tensor_tensor(
                    out=t2[:], in0=ps_v[:], scalar=vcol, in1=t1[:],
                    op0=Alu.add, op1=Alu.max,
                )
                eT = work.tile([H, B * P], F32, tag="eT")
                nc.scalar.activation(eT[:], t2[:], Act.Exp)
                ebst["e"] = eT
                # b=1 broadcast via replicate DMA (latency hidden by b0 work)
                src = eT[:, P : 2 * P]
                rep = bass.AP(
                    tensor=src.tensor, offset=src.offset,
                    ap=[list(src.ap[0]), [0, 4], list(src.ap[1])],
                )
                nc.sync.dma_start(out=eb1_sb[:], in_=rep)
                ebst[1] = eb1_sb

            def eb_mm():
                # b=0 broadcast eb[q, I] = e[q//4, I] via indicator matmul
                ps_eb = psE.tile([P, P], F32, tag="eb", name="eb0")
                nc.tensor.matmul(ps_eb[:], ind, ebst["e"][:, 0:P])
                ebst[0] = ps_eb

            aggst = {}

            def agg_step(b, tk):
                if tk == 0:
                    aggst[b] = (
                        psA.tile([C1, P], F32, tag="pse", name=f"pse{b}"),
                        psA.tile([C1, P], F32, tag="pso", name=f"pso{b}"),
                    )
                wtile = work.tile([P, P], F32, tag="wtile")
                nc.vector.scalar_tensor_tensor(
                    out=wtile[:], in0=dT_sb[:, tk, :], scalar=THR,
                    in1=ebst[b][:], op0=Alu.is_lt, op1=Alu.mult,
                )
                ps_g = aggst[b][tk % 2]
                nc.tensor.matmul(
                    ps_g[:],
                    xa_sb[:, b, C1 * tk : C1 * tk + C1],
                    wtile[:],
                    start=(tk < 2),
                    stop=(tk >= NJ - 2),
                    skip_group_check=True,
                )

            def finalize(b):
                ps_even, ps_odd = aggst[b]
                gtmp = work.tile([C1, P], F32, tag="gtmp")
                nc.vector.tensor_copy(gtmp[:], ps_odd[:])
                gt = work.tile([C1, P], F32, tag="gt")
                nc.vector.tensor_add(out=gt[:], in0=ps_even[:], in1=gtmp[:])
                ps_f = psT.tile([P, H1], F32, tag="ps", name=f"psf{b}")
                nc.tensor.matmul(ps_f[:], gt[:], wfin)
                ot = work.tile([P, H1], F32, tag="ot", name=f"ot{b}")
                nc.vector.tensor_copy(ot[:], ps_f[:])
                eng = nc.sync if b == 0 else nc.scalar
                eng.dma_start(out=out_d[b], in_=ot[:])

            # ---- interleaved emission ----
            v_steps()
            eb_mm()
            agg_step(0, 0)
            agg_step(0, 1)
            agg_step(0, 2)
            agg_step(0, 3)
            agg_step(0, 4)
            agg_step(0, 5)
            agg_step(0, 6)
            agg_step(0, 7)
            agg_step(1, 0)
            agg_step(1, 1)
            finalize(0)
            for tk in range(2, NJ):
                agg_step(1, tk)
            finalize(1)

    nc.finalize()
    return nc


def kernel(x, dist_mat, proj_w, proj_b, a_w, trace=False):
    global LAST_RESULT
    from concourse.bass_utils import run_bass_kernel_spmd

    x = np.ascontiguousarray(np.asarray(x, dtype=np.float32))
    dist_mat = np.asarray(dist_mat, dtype=np.float32)
    proj_w = np.asarray(proj_w, dtype=np.float32)
    proj_b = np.asarray(proj_b, dtype=np.float32).reshape(H)
    a_w = np.asarray(a_w, dtype=np.float32).reshape(2 * H)

    if "nc" not in _CACHE:
        _CACHE["nc"] = _build()
    nc = _CACHE["nc"]

    # ---- host-side constant folding (all tiny) ----
    a1, a2 = a_w[:H], a_w[H:]
    s1 = np.float32(a1.sum(dtype=np.float32))
    m32 = s1 * np.eye(H, dtype=np.float32) + a2[:, None]  # v = m32.T @ hT
    wta = proj_w.T.astype(np.float32)                     # [C, H]
    wm = np.ascontiguousarray(wta @ m32)                  # fold h->v projection
    vcol = np.ascontiguousarray((m32.T @ proj_b).reshape(H, 1))
    # final projection [x | 1] -> [h | 1]: bias row, ones column slot
    wfin = np.zeros((C1, H1), np.float32)
    wfin[:C, :H] = wta
    wfin[C, :H] = proj_b
    wfin[C, H] = 1.0
    # eb indicator: ind[k, q] = 1 iff q//4 == k
    ind = np.zeros((H, P), np.float32)
    for k in range(H):
        ind[k, 4 * k : 4 * k + 4] = 1.0
    # pack all the small constants into one block -> a single DMA
    cblk0 = np.zeros((P, 256), np.float32)
    cblk0[0:C, 0:H] = wm
    cblk0[0:H, H : H + 1] = vcol
    cblk0[0:H, 64:192] = ind
    cblk0[0:C1, 192 : 192 + H1] = wfin

    dist_fixed = dist_mat.copy()
    np.fill_diagonal(dist_fixed, 0.0)  # adj diagonal forced to 1
    distT = np.ascontiguousarray(dist_fixed.T)

    # token J = 8*q + tk; row q of xa[b] holds tokens 8q..8q+7, each
    # augmented with a trailing 1.0 (for the softmax denominator)
    xa = np.ones((B, N, C1), np.float32)
    xa[:, :, :C] = x
    xa = xa.reshape(B, P, NJ * C1)

    in_maps = []
    for c in range(NCORES):
        sl = slice(c * P, (c + 1) * P)
        in_maps.append(
            {
                "xa": xa,
                "xoT": np.concatenate([x[0, sl, :].T, x[1, sl, :].T], axis=1),
                "distT": np.ascontiguousarray(distT[:, sl]),
                "cblk": cblk0,
            }
        )

    res = run_bass_kernel_spmd(nc, in_maps, core_ids=list(range(NCORES)), trace=trace)
    LAST_RESULT = res
    full = np.concatenate([res.results[c]["out"] for c in range(NCORES)], axis=1)
    return np.ascontiguousarray(full[:, :, :H] / full[:, :, H : H + 1])
